# revision 1
# baseline (speedup 1.0000x reference)
"""Collective variant: K/V projection split across core pairs + pair AllGather.

Core c = (batch c//2, stripe h = c%2). Each core projects K^T and V only for
its own key half (s in [h*1024, (h+1)*1024)), then the pair exchanges halves
via two AllGathers (one per 512-key own-block) so attention can start as soon
as the first halves have been gathered.

Gathered DRAM layout (per 512-key global block b, r = b//2 = producing rank,
sub = b%2 selects which of the two collectives):
  cc = ccA if b%2==0 else ccB; base = r*2048
  KT tile k:  cc[base + k*128 : +128, :]                       [128, 512]
  V tile st:  cc[base + 1024 + st*256 : +256, :] as [128,1024] (row-pair fold)
"""

import numpy as np

B, S, E, KD = 4, 2048, 1024, 1024
NCORES = 8
P = 128
ET = E // P
KT = KD // P
NQT = 8
NBLK = 4
NEG = -30000.0
SCALE = 1.0 / float(np.sqrt(KD))

_prog_cache = {}


def _n_blocks(t):
    return (t + 2) // 2


def _build_body(ctx, tc, ap):
    from concourse import mybir
    from concourse.masks import make_identity

    nc = tc.nc
    f32 = mybir.dt.float32
    f32r = mybir.dt.float32r
    Exp = mybir.ActivationFunctionType.Exp
    X = mybir.AxisListType.X

    xTq_t = ap["xTq"].rearrange("(t p) q -> t p q", p=P)    # [8, 128, 1024]
    xTp_t = ap["xTp"].rearrange("(t p) s -> t p s", p=P)    # [8, 128, 1024]
    wqT_t = ap["wqT"].rearrange("(t p) k -> t p k", p=P)
    wkT_t = ap["wkT"].rearrange("(t p) k -> t p k", p=P)
    wvT_t = ap["wvT"].rearrange("(t p) f -> t p f", p=P)
    out_t = ap["out"].rearrange("(t p) f -> t p f", p=P)

    # ---- persistent tiles
    qt_pool = ctx.enter_context(tc.tile_pool(name="qt", bufs=1))
    QT = [qt_pool.tile([P, 1024], f32r, name=f"qt{k}", tag=f"qt{k}") for k in range(KT)]
    acc_pool = ctx.enter_context(tc.tile_pool(name="acc", bufs=1))
    OACC = [acc_pool.tile([P, E], f32, name=f"oacc{t}", tag=f"oacc{t}") for t in range(NQT)]
    RS = [acc_pool.tile([P, NBLK], f32, name=f"rs{t}", tag=f"rs{t}") for t in range(NQT)]
    const_pool = ctx.enter_context(tc.tile_pool(name="const", bufs=1))
    fin_pool = ctx.enter_context(tc.tile_pool(name="fin", bufs=4))

    # ---- DRAM tiles for the pair exchange
    dram = ctx.enter_context(tc.tile_pool(name="dram", bufs=1, space="DRAM"))
    ccin = [dram.tile([2048, 512], f32r, name=f"ccin{i}", tag=f"ccin{i}") for i in range(2)]
    ccout = [dram.tile([4096, 512], f32r, name=f"ccout{i}", tag=f"ccout{i}") for i in range(2)]

    # ---- PSUM: pp (projection evict) lives only through the projection
    # phases; its banks are then handed to the attention pools (vp bufs=2).
    pp_ctx = tc.tile_pool(name="pp", bufs=4, space="PSUM")
    pp = pp_ctx.__enter__()

    # ---- Phase A: own-half K/V projection + pair exchange.
    # Emitted FIRST so the K/V inputs arrive first and the collectives launch
    # as early as possible; the QT projection then runs underneath the
    # collective latency instead of in front of it.
    with tc.tile_pool(name="wkp", bufs=1) as wk_pool, \
         tc.tile_pool(name="wvp", bufs=1) as wv_pool, \
         tc.tile_pool(name="xpp", bufs=1) as xp_pool, \
         tc.tile_pool(name="stg", bufs=2) as stg_pool:
        wk = [wk_pool.tile([P, KD], f32r, name=f"wk{e}", tag=f"wk{e}") for e in range(ET)]
        xp = [xp_pool.tile([P, 1024], f32r, name=f"xp{e}", tag=f"xp{e}") for e in range(ET)]
        wv = [wv_pool.tile([P, E], f32r, name=f"wv{e}", tag=f"wv{e}") for e in range(ET)]
        # arrival order tuned to keep PE continuously fed:
        # [xp0+wk] -> xp1 -> wv -> (wq, xq emitted in phase B)
        for e in range(ET):
            nc.sync.dma_start(out=xp[e], in_=xTp_t[e])
            nc.sync.dma_start(out=wk[e], in_=wkT_t[e])
        for e in range(ET):
            nc.sync.dma_start(out=wv[e], in_=wvT_t[e])

        for ob in range(2):
            for k in range(KT):
                ps = pp.tile([P, 512], f32, name="ps_kt", tag="pp")
                for e in range(ET):
                    nc.tensor.matmul(ps, wk[e][:, k * P:(k + 1) * P],
                                     xp[e][:, ob * 512:(ob + 1) * 512],
                                     start=(e == 0), stop=(e == ET - 1))
                ko = stg_pool.tile([P, 512], f32r, name="ko", tag="ko", bufs=3)
                nc.vector.tensor_copy(ko, ps)
                nc.gpsimd.dma_start(out=ccin[ob][k * P:(k + 1) * P, :], in_=ko)
            # V_own[ob]: [512, 1024] -> rows 1024: as [1024, 512] row-pair fold
            for st in range(4):
                vo = stg_pool.tile([P, E], f32r, name="vo", tag="vo", bufs=3)
                for fb in range(2):
                    ps = pp.tile([P, 512], f32, name="ps_v", tag="pp")
                    for e in range(ET):
                        nc.tensor.matmul(
                            ps, xp[e][:, ob * 512 + st * P: ob * 512 + (st + 1) * P],
                            wv[e][:, fb * 512:(fb + 1) * 512],
                            start=(e == 0), stop=(e == ET - 1))
                    if fb == 0:
                        nc.scalar.copy(vo[:, fb * 512:(fb + 1) * 512], ps)
                    else:
                        nc.vector.tensor_copy(vo[:, fb * 512:(fb + 1) * 512], ps)
                vdst = ccin[ob][1024 + st * 256: 1024 + (st + 1) * 256, :]
                nc.gpsimd.dma_start(
                    out=vdst.rearrange("(s a) c -> s (a c)", a=2), in_=vo)
            nc.gpsimd.collective_compute(
                "AllGather", mybir.AluOpType.bypass,
                replica_groups=[[0, 1], [2, 3], [4, 5], [6, 7]],
                ins=[ccin[ob].opt()], outs=[ccout[ob].opt()],
            )

    # ---- Phase B: QT[k, q] projection (runs while the collectives fly)
    with tc.tile_pool(name="wqp", bufs=1) as wq_pool, \
         tc.tile_pool(name="xqp", bufs=1) as xq_pool:
        wq = [wq_pool.tile([P, KD], f32r, name=f"wq{e}", tag=f"wq{e}") for e in range(ET)]
        xq = [xq_pool.tile([P, 1024], f32r, name=f"xq{e}", tag=f"xq{e}") for e in range(ET)]
        for e in range(ET):
            nc.sync.dma_start(out=wq[e], in_=wqT_t[e])
            nc.sync.dma_start(out=xq[e], in_=xTq_t[e])
        for qb in range(2):
            for k in range(KT):
                ps = pp.tile([P, 512], f32, name="ps_qt", tag="pp")
                for e in range(ET):
                    nc.tensor.matmul(
                        ps, wq[e][:, k * P:(k + 1) * P],
                        xq[e][:, qb * 512:(qb + 1) * 512],
                        start=(e == 0), stop=(e == ET - 1))
                if k % 2 == 0:
                    nc.vector.tensor_copy(QT[k][:, qb * 512:(qb + 1) * 512], ps)
                else:
                    nc.scalar.copy(QT[k][:, qb * 512:(qb + 1) * 512], ps)

    # ---- Phase C: attention over global blocks
    pp_ctx.__exit__(None, None, None)
    cm = const_pool.tile([P, 256], f32, name="cm")
    nc.sync.dma_start(out=cm, in_=ap["cmask"])
    ident_f32 = const_pool.tile([P, P], f32, name="ident_f32")
    make_identity(nc, ident_f32)
    ident = const_pool.tile([P, P], f32r, name="ident")
    nc.vector.tensor_copy(ident, ident_f32)
    sp = ctx.enter_context(tc.tile_pool(name="sp", bufs=2, space="PSUM"))
    tp = ctx.enter_context(tc.tile_pool(name="tp", bufs=2, space="PSUM"))
    vp = ctx.enter_context(tc.tile_pool(name="vp", bufs=2, space="PSUM"))
    kt_pool = ctx.enter_context(tc.tile_pool(name="ktp", bufs=2))
    vb_pool = ctx.enter_context(tc.tile_pool(name="vbp", bufs=2))
    p_pool = ctx.enter_context(tc.tile_pool(name="ppb", bufs=4))
    pt_pool = ctx.enter_context(tc.tile_pool(name="ptp", bufs=6))

    ORDER = (0, 2, 1, 3)  # blocks 0,2 come from CC1 — start before CC2 lands
    last_visit = {t: [b for b in ORDER if t >= 2 * b][-1] for t in range(NQT)}

    def emit_pv(pend):
        # deferred transpose/copy/PV/accumulate for one (blk, t) work item;
        # runs one position behind the scores stream so the PE->DVE->PE
        # transpose-copy chain and exp latency hide behind matmul work.
        pb, w, blk, t, vbt = pend
        nst = w // P
        tpss = []
        for st in range(nst):
            tps = tp.tile([P, P], f32r, name="tps", tag="tp")
            nc.tensor.transpose(tps, pb[:, st * P:(st + 1) * P], ident)
            tpss.append(tps)
            if st > 0:
                pts = pt_pool.tile([P, P], f32r, name="pts", tag=f"pt{st-1}")
                nc.vector.tensor_copy(pts, tpss[st - 1])
                tpss[st - 1] = pts
        pts = pt_pool.tile([P, P], f32r, name="pts", tag=f"pt{nst-1}")
        nc.vector.tensor_copy(pts, tpss[nst - 1])
        tpss[nst - 1] = pts
        vps = [vp.tile([P, 512], f32, name=f"vps{fb}", tag=f"vp{fb}") for fb in range(2)]
        for st in range(nst):
            for fb in range(2):
                nc.tensor.matmul(vps[fb], tpss[st],
                                 vbt[st][:, fb * 512:(fb + 1) * 512],
                                 start=(st == 0), stop=(st == nst - 1))
        for fb in range(2):
            dst = OACC[t][:, fb * 512:(fb + 1) * 512]
            if blk == 0:
                nc.vector.tensor_copy(dst, vps[fb])
            else:
                nc.vector.tensor_add(dst, dst, vps[fb])
        if blk == last_visit[t]:
            nb = _n_blocks(t)
            rsum = fin_pool.tile([P, 1], f32, name="rsum", tag="rsum")
            nc.vector.reduce_sum(rsum, RS[t][:, :nb], axis=X)
            rinv = fin_pool.tile([P, 1], f32, name="rinv", tag="rinv")
            nc.vector.reciprocal(rinv, rsum)
            nc.scalar.activation(OACC[t], OACC[t],
                                 mybir.ActivationFunctionType.Copy, scale=rinv)
            nc.sync.dma_start(out=out_t[t], in_=OACC[t])

    pending = None  # pipeline carries across block boundaries (vb bufs=2)
    for blk in ORDER:
        r, sub = blk // 2, blk % 2
        cc = ccout[sub]
        base = r * 2048
        ktb = [kt_pool.tile([P, 512], f32r, name=f"ktb{k}", tag=f"ktb{k}") for k in range(KT)]
        for k in range(KT):
            nc.sync.dma_start(out=ktb[k], in_=cc[base + k * P: base + (k + 1) * P, :])
        vbt = [vb_pool.tile([P, E], f32r, name=f"vb{st}", tag=f"vb{st}") for st in range(4)]
        for st in range(4):
            vsrc = cc[base + 1024 + st * 256: base + 1024 + (st + 1) * 256, :]
            nc.sync.dma_start(out=vbt[st], in_=vsrc.rearrange("(s a) c -> s (a c)", a=2))

        for t in range(2 * blk, NQT):
            w = min(512, 256 * (t + 1) - 512 * blk)
            is_diag = (blk == _n_blocks(t) - 1)
            sps = sp.tile([P, 512], f32, name="sps", tag="sp")
            for k in range(KT):
                nc.tensor.matmul(sps[:, :w], QT[k][:, t * P:(t + 1) * P],
                                 ktb[k][:, :w], start=(k == 0), stop=(k == KT - 1))
            if is_diag:
                nc.vector.tensor_add(sps[:, w - 256:w], sps[:, w - 256:w], cm)
            pb = p_pool.tile([P, 512], f32r, name="pb", tag="pb")
            nc.scalar.activation(pb[:, :w], sps[:, :w], Exp, scale=SCALE,
                                 accum_out=RS[t][:, blk:blk + 1])
            if pending is not None:
                emit_pv(pending)
            pending = (pb, w, blk, t, vbt)
    emit_pv(pending)


def build_program():
    if "nc" in _prog_cache:
        return _prog_cache["nc"]
    from contextlib import ExitStack
    from concourse import bacc, mybir
    import concourse.tile as tile

    nc = bacc.Bacc("TRN2", target_bir_lowering=False, debug=False,
                   num_devices=NCORES)
    f32 = mybir.dt.float32
    f32r = mybir.dt.float32r
    ap = {
        "xTq": nc.dram_tensor("xTq", [E, 1024], f32r, kind="ExternalInput").ap(),
        "xTp": nc.dram_tensor("xTp", [E, 1024], f32r, kind="ExternalInput").ap(),
        "wqT": nc.dram_tensor("wqT", [E, KD], f32r, kind="ExternalInput").ap(),
        "wkT": nc.dram_tensor("wkT", [E, KD], f32r, kind="ExternalInput").ap(),
        "wvT": nc.dram_tensor("wvT", [E, E], f32r, kind="ExternalInput").ap(),
        "cmask": nc.dram_tensor("cmask", [P, 256], f32, kind="ExternalInput").ap(),
        "out": nc.dram_tensor("out", [1024, E], f32, kind="ExternalOutput").ap(),
    }
    with tile.TileContext(nc) as tc:
        with ExitStack() as ctx:
            _build_body(ctx, tc, ap)
    nc.compile()
    _prog_cache["nc"] = nc
    return nc


def make_in_maps(x, W_q, W_k, W_v):
    x = np.ascontiguousarray(np.asarray(x, np.float32))
    wqT = np.ascontiguousarray(np.asarray(W_q, np.float32).T)
    wkT = np.ascontiguousarray(np.asarray(W_k, np.float32).T)
    wvT = np.ascontiguousarray(np.asarray(W_v, np.float32).T)
    i = np.arange(P)[:, None]
    j = np.arange(256)[None, :]
    cmasks = [np.where(j <= i + 128, 0.0, NEG).astype(np.float32),
              np.where(j <= i, 0.0, NEG).astype(np.float32)]
    in_maps = []
    for c in range(NCORES):
        b, h = c // 2, c % 2
        xT = np.ascontiguousarray(x[b].T)
        qtiles = [2 * t + (1 - h) for t in range(NQT)]
        qcols = np.concatenate([np.arange(g * P, (g + 1) * P) for g in qtiles])
        xTq = np.ascontiguousarray(xT[:, qcols])
        xTp = np.ascontiguousarray(xT[:, h * 1024:(h + 1) * 1024])
        in_maps.append({
            "xTq": xTq, "xTp": xTp, "wqT": wqT, "wkT": wkT, "wvT": wvT,
            "cmask": cmasks[h],
        })
    return in_maps


def assemble(results):
    out = np.zeros((B, S, E), np.float32)
    for c in range(NCORES):
        b, h = c // 2, c % 2
        co = results[c]["out"]
        for t in range(NQT):
            g = 2 * t + (1 - h)
            out[b, g * P:(g + 1) * P, :] = co[t * P:(t + 1) * P]
    return out


def kernel(x, W_q, W_k, W_v):
    from concourse.bass_utils import run_bass_kernel_spmd
    nc = build_program()
    in_maps = make_in_maps(x, W_q, W_k, W_v)
    res = run_bass_kernel_spmd(nc, in_maps, core_ids=list(range(NCORES)))
    return assemble(res.results)



# revision 2
# speedup vs baseline: 2.3696x; 2.3696x over previous
"""No-collective causal-attention kernel: full local K/V recompute per core.

Core c = (batch c//2, parity h = c%2). Each core owns the interleaved query
tiles g = 2t + (1-h) (t = 0..7 local) of its batch, projects Q for those
rows, and projects the FULL K^T and V locally (no pair exchange) so the 8
cores run fully independent SPMD programs. The ~2.1G extra MACs per core are
far cheaper than the collective they replace.

Attention uses a transposed-scores formulation: S^T[s, q] tiles come straight
out of the scores matmul with keys on partitions, so exp(S^T) = P^T is
directly the stationary operand for the P@V matmuls — no PE transposes, no
PSUM->SBUF transpose copies. Softmax denominators are computed with N=1
matmuls against a ones vector (sum over the key/partition axis).

All operands are stored bf16 (fp32 PSUM accumulation), halving DMA and SBUF
and keeping every matmul at 1 cycle/row even at N=128.
"""

import numpy as np

B, S, E, KD = 4, 2048, 1024, 1024
NCORES = 8
P = 128
ET = E // P          # 8 e-tiles of the contraction dim
KT = KD // P         # 8 kd-tiles
NQT = 8              # local query tiles per core (128 rows each)
NBLK = 4             # 512-key blocks
NEG = -30000.0
SCALE = 1.0 / float(np.sqrt(KD))

_prog_cache = {}


def _build_body(ctx, tc, ap):
    from concourse import mybir

    nc = tc.nc
    f32 = mybir.dt.float32
    bf16 = mybir.dt.bfloat16
    Exp = mybir.ActivationFunctionType.Exp
    Copy = mybir.ActivationFunctionType.Copy
    X = mybir.AxisListType.X

    xq_t = ap["xq"].rearrange("(t p) q -> t p q", p=P)    # [8, 128, 1024]
    xp_t = ap["xp"].rearrange("(t p) s -> t p s", p=P)    # [8, 128, 2048]
    wq_t = ap["wq"].rearrange("(t p) k -> t p k", p=P)    # [8, 128, 1024]
    wk_t = ap["wk"].rearrange("(t p) k -> t p k", p=P)
    wv_t = ap["wv"].rearrange("(t p) f -> t p f", p=P)
    out_t = ap["out"].rearrange("(t p) f -> t p f", p=P)

    # ---- persistent SBUF tiles
    qt_pool = ctx.enter_context(tc.tile_pool(name="qt", bufs=1))
    QT = [qt_pool.tile([P, 1024], bf16, name=f"qt{k}", tag=f"qt{k}") for k in range(KT)]
    acc_pool = ctx.enter_context(tc.tile_pool(name="acc", bufs=1))
    OACC = [acc_pool.tile([P, E], f32, name=f"oacc{t}", tag=f"oacc{t}") for t in range(NQT)]
    RS = [acc_pool.tile([P, NBLK], f32, name=f"rs{t}", tag=f"rs{t}") for t in range(NQT)]
    const_pool = ctx.enter_context(tc.tile_pool(name="const", bufs=1))
    fin_pool = ctx.enter_context(tc.tile_pool(name="fin", bufs=4))

    cm = const_pool.tile([P, 256], f32, name="cm")
    nc.sync.dma_start(out=cm, in_=ap["cmaskT"])
    ones = const_pool.tile([P, 1], bf16, name="ones")
    nc.sync.dma_start(out=ones, in_=ap["ones"])

    # full-x tiles stay alive through the last V projection
    xp_pool = ctx.enter_context(tc.tile_pool(name="xpp", bufs=1))
    xp = [xp_pool.tile([P, 2048], bf16, name=f"xp{e}", tag=f"xp{e}") for e in range(ET)]
    wk_pool = ctx.enter_context(tc.tile_pool(name="wkp", bufs=1))
    wk = [wk_pool.tile([P, KD], bf16, name=f"wk{e}", tag=f"wk{e}") for e in range(ET)]
    wv_pool = ctx.enter_context(tc.tile_pool(name="wvp", bufs=1))
    wv = [wv_pool.tile([P, E], bf16, name=f"wv{e}", tag=f"wv{e}") for e in range(ET)]

    # ---- PSUM pools
    pp = ctx.enter_context(tc.tile_pool(name="pp", bufs=2, space="PSUM"))
    sp = ctx.enter_context(tc.tile_pool(name="sp", bufs=2, space="PSUM"))
    vp = ctx.enter_context(tc.tile_pool(name="vp", bufs=1, space="PSUM"))
    dn = ctx.enter_context(tc.tile_pool(name="dn", bufs=2, space="PSUM"))

    # ---- Phase A: Q projection (QT[k] = [128 kd, 1024 q], bf16)
    with tc.tile_pool(name="wqp", bufs=1) as wq_pool, \
         tc.tile_pool(name="xqp", bufs=1) as xq_pool:
        wq = [wq_pool.tile([P, KD], bf16, name=f"wq{e}", tag=f"wq{e}") for e in range(ET)]
        xq = [xq_pool.tile([P, 1024], bf16, name=f"xq{e}", tag=f"xq{e}") for e in range(ET)]
        # interleave so the e-loop of the first matmuls chases the DMAs
        for e in range(ET):
            nc.sync.dma_start(out=xq[e], in_=xq_t[e])
            nc.sync.dma_start(out=wq[e], in_=wq_t[e])
        for e in range(ET):
            nc.gpsimd.dma_start(out=xp[e], in_=xp_t[e])
            nc.gpsimd.dma_start(out=wk[e], in_=wk_t[e])
        for e in range(ET):
            nc.gpsimd.dma_start(out=wv[e], in_=wv_t[e])

        for qb in range(2):
            for k in range(KT):
                ps = pp.tile([P, 512], f32, name="ps_q", tag="pp")
                for e in range(ET):
                    nc.tensor.matmul(ps, wq[e][:, k * P:(k + 1) * P],
                                     xq[e][:, qb * 512:(qb + 1) * 512],
                                     start=(e == 0), stop=(e == ET - 1))
                if k % 2 == 0:
                    nc.vector.tensor_copy(QT[k][:, qb * 512:(qb + 1) * 512], ps)
                else:
                    nc.scalar.copy(QT[k][:, qb * 512:(qb + 1) * 512], ps)

    # ---- per-block KT/V pools (double-buffered across blocks)
    kt_pool = ctx.enter_context(tc.tile_pool(name="ktp", bufs=2))
    vb_pool = ctx.enter_context(tc.tile_pool(name="vbp", bufs=2))
    pt_pool = ctx.enter_context(tc.tile_pool(name="ptp", bufs=3))

    last_visit = {t: t // 2 for t in range(NQT)}

    def emit_pv(pend):
        # one item behind the scores stream so exp latency hides under PE work
        pb, w, blk, t, vbt = pend
        nst = w // P
        # softmax denominator: sum over keys (partition axis) via N=1 matmuls
        dps = dn.tile([P, 1], f32, name="dps", tag="dn")
        for st in range(nst):
            nc.tensor.matmul(dps, pb[:, st * P:(st + 1) * P], ones,
                             start=(st == 0), stop=(st == nst - 1))
        nc.vector.tensor_copy(RS[t][:, blk:blk + 1], dps)
        vps = [vp.tile([P, 512], f32, name=f"vps{fb}", tag=f"vp{fb}") for fb in range(2)]
        for st in range(nst):
            for fb in range(2):
                nc.tensor.matmul(vps[fb], pb[:, st * P:(st + 1) * P],
                                 vbt[st][:, fb * 512:(fb + 1) * 512],
                                 start=(st == 0), stop=(st == nst - 1))
        for fb in range(2):
            dst = OACC[t][:, fb * 512:(fb + 1) * 512]
            if blk == 0:
                nc.vector.tensor_copy(dst, vps[fb])
            else:
                nc.vector.tensor_add(dst, dst, vps[fb])
        if blk == last_visit[t]:
            nb = blk + 1
            rsum = fin_pool.tile([P, 1], f32, name="rsum", tag="rsum")
            nc.vector.reduce_sum(rsum, RS[t][:, :nb], axis=X)
            rinv = fin_pool.tile([P, 1], f32, name="rinv", tag="rinv")
            nc.vector.reciprocal(rinv, rsum)
            nc.scalar.activation(OACC[t], OACC[t], Copy, scale=rinv)
            nc.sync.dma_start(out=out_t[t], in_=OACC[t])

    pending = None
    for blk in range(NBLK):
        # K^T projection for this block: ktb[k] = [128 kd, 512 s]
        ktb = [kt_pool.tile([P, 512], bf16, name=f"ktb{k}", tag=f"ktb{k}")
               for k in range(KT)]
        for k in range(KT):
            ps = pp.tile([P, 512], f32, name="ps_kt", tag="pp")
            for e in range(ET):
                nc.tensor.matmul(ps, wk[e][:, k * P:(k + 1) * P],
                                 xp[e][:, blk * 512:(blk + 1) * 512],
                                 start=(e == 0), stop=(e == ET - 1))
            if k % 2 == 0:
                nc.vector.tensor_copy(ktb[k], ps)
            else:
                nc.scalar.copy(ktb[k], ps)
        # V projection for this block: vbt[st] = [128 s, 1024 f]
        vbt = [vb_pool.tile([P, E], bf16, name=f"vb{st}", tag=f"vb{st}")
               for st in range(4)]
        for st in range(4):
            for fb in range(2):
                ps = pp.tile([P, 512], f32, name="ps_v", tag="pp")
                for e in range(ET):
                    nc.tensor.matmul(
                        ps, xp[e][:, blk * 512 + st * P: blk * 512 + (st + 1) * P],
                        wv[e][:, fb * 512:(fb + 1) * 512],
                        start=(e == 0), stop=(e == ET - 1))
                if fb == 0:
                    nc.scalar.copy(vbt[st][:, fb * 512:(fb + 1) * 512], ps)
                else:
                    nc.vector.tensor_copy(vbt[st][:, fb * 512:(fb + 1) * 512], ps)

        # attention items for this block
        for t in range(2 * blk, NQT):
            w = min(512, 256 * (t + 1) - 512 * blk)
            nst = w // P
            is_diag = (blk == last_visit[t])
            sps = sp.tile([P, 512], f32, name="sps", tag="sp")
            for st in range(nst):
                dst = sps[:, st * P:(st + 1) * P]
                for k in range(KT):
                    nc.tensor.matmul(dst, ktb[k][:, st * P:(st + 1) * P],
                                     QT[k][:, t * P:(t + 1) * P],
                                     start=(k == 0), stop=(k == KT - 1))
            if is_diag:
                nc.vector.tensor_add(sps[:, w - 256:w], sps[:, w - 256:w], cm)
            pb = pt_pool.tile([P, 512], bf16, name="pb", tag="pb")
            nc.scalar.activation(pb[:, :w], sps[:, :w], Exp, scale=SCALE)
            if pending is not None:
                emit_pv(pending)
            pending = (pb, w, blk, t, vbt)
    emit_pv(pending)


def build_program():
    if "nc" in _prog_cache:
        return _prog_cache["nc"]
    from contextlib import ExitStack
    from concourse import bacc, mybir
    import concourse.tile as tile

    nc = bacc.Bacc("TRN2", target_bir_lowering=False, debug=False,
                   num_devices=NCORES)
    f32 = mybir.dt.float32
    bf16 = mybir.dt.bfloat16
    ap = {
        "xq": nc.dram_tensor("xq", [E, 1024], bf16, kind="ExternalInput").ap(),
        "xp": nc.dram_tensor("xp", [E, 2048], bf16, kind="ExternalInput").ap(),
        "wq": nc.dram_tensor("wq", [E, KD], bf16, kind="ExternalInput").ap(),
        "wk": nc.dram_tensor("wk", [E, KD], bf16, kind="ExternalInput").ap(),
        "wv": nc.dram_tensor("wv", [E, E], bf16, kind="ExternalInput").ap(),
        "cmaskT": nc.dram_tensor("cmaskT", [P, 256], f32, kind="ExternalInput").ap(),
        "ones": nc.dram_tensor("ones", [P, 1], bf16, kind="ExternalInput").ap(),
        "out": nc.dram_tensor("out", [1024, E], f32, kind="ExternalOutput").ap(),
    }
    with tile.TileContext(nc) as tc:
        with ExitStack() as ctx:
            _build_body(ctx, tc, ap)
    nc.compile()
    _prog_cache["nc"] = nc
    return nc


def make_in_maps(x, W_q, W_k, W_v):
    from concourse import mybir
    bf16 = mybir.dt.np(mybir.dt.bfloat16)
    x = np.asarray(x, np.float32)
    wqT = np.ascontiguousarray(np.asarray(W_q, np.float32).T).astype(bf16)
    wkT = np.ascontiguousarray(np.asarray(W_k, np.float32).T).astype(bf16)
    wvT = np.ascontiguousarray(np.asarray(W_v, np.float32).T).astype(bf16)
    # transposed-layout causal masks for the last 256 keys of the diagonal
    # block: maskT[p, st*128 + q] with s_local = st*128 + p, unmasked iff
    # s_local <= q + 128 (h=0, g odd) / s_local <= q (h=1, g even)
    p = np.arange(P)[:, None]
    q = np.arange(P)[None, :]
    def mk(thresh_extra):
        m0 = np.where(p <= q + thresh_extra, 0.0, NEG)          # st 0
        m1 = np.where(p + 128 <= q + thresh_extra, 0.0, NEG)    # st 1
        return np.concatenate([m0, m1], axis=1).astype(np.float32)
    cmasksT = [mk(128), mk(0)]
    ones = np.ones((P, 1), dtype=bf16)
    in_maps = []
    for c in range(NCORES):
        b, h = c // 2, c % 2
        xT = np.ascontiguousarray(x[b].T)
        qtiles = [2 * t + (1 - h) for t in range(NQT)]
        qcols = np.concatenate([np.arange(g * P, (g + 1) * P) for g in qtiles])
        xq = np.ascontiguousarray(xT[:, qcols]).astype(bf16)
        xp = xT.astype(bf16)
        in_maps.append({
            "xq": xq, "xp": xp, "wq": wqT, "wk": wkT, "wv": wvT,
            "cmaskT": cmasksT[h], "ones": ones,
        })
    return in_maps


def assemble(results):
    out = np.zeros((B, S, E), np.float32)
    for c in range(NCORES):
        b, h = c // 2, c % 2
        co = results[c]["out"]
        for t in range(NQT):
            g = 2 * t + (1 - h)
            out[b, g * P:(g + 1) * P, :] = co[t * P:(t + 1) * P]
    return out


def kernel(x, W_q, W_k, W_v):
    from concourse.bass_utils import run_bass_kernel_spmd
    nc = build_program()
    in_maps = make_in_maps(x, W_q, W_k, W_v)
    res = run_bass_kernel_spmd(nc, in_maps, core_ids=list(range(NCORES)))
    return assemble(res.results)


# revision 40
# speedup vs baseline: 2.6073x; 1.1003x over previous
"""No-collective causal-attention kernel: full local K/V recompute per core.

Core c = (batch c//2, parity h = c%2). Each core owns the interleaved query
tiles g = 2t + (1-h) (t = 0..7 local) of its batch, projects Q for those
rows, and projects the FULL K^T and V locally (no pair exchange) so the 8
cores run fully independent SPMD programs. The ~2.1G extra MACs per core are
far cheaper than the collective they replace.

Attention uses a transposed-scores formulation: S^T[s, q] tiles come straight
out of the scores matmul with keys on partitions, so exp(S^T) = P^T is
directly the stationary operand for the P@V matmuls — no PE transposes, no
PSUM->SBUF transpose copies. Softmax denominators are computed with N=1
matmuls against a ones vector (sum over the key/partition axis).

All operands are stored bf16 (fp32 PSUM accumulation), halving DMA and SBUF
and keeping every matmul at 1 cycle/row even at N=128.
"""

import numpy as np

B, S, E, KD = 4, 2048, 1024, 1024
NCORES = 8
P = 128
ET = E // P          # 8 e-tiles of the contraction dim
KT = KD // P         # 8 kd-tiles
NQT = 8              # local query tiles per core (128 rows each)
NBLK = 4             # 512-key blocks
NEG = -30000.0
SCALE = 1.0 / float(np.sqrt(KD))

_prog_cache = {}


def _build_body(ctx, tc, ap):
    from concourse import mybir

    nc = tc.nc
    f32 = mybir.dt.float32
    bf16 = mybir.dt.bfloat16
    Exp = mybir.ActivationFunctionType.Exp
    Copy = mybir.ActivationFunctionType.Copy
    X = mybir.AxisListType.X

    wkxa_t = ap["wkxa"].rearrange("(t p) c -> t p c", p=P)    # [8, 128, 1024]
    wkxb_t = ap["wkxb"].rearrange("(t p) c -> t p c", p=P)    # [8, 128, 512]
    xqwqa_t = ap["xqwqa"].rearrange("(t p) c -> t p c", p=P)  # [8, 128, 1024]
    xqwqb_t = ap["xqwqb"].rearrange("(t p) c -> t p c", p=P)  # [8, 128, 1024]
    xbt_t = ap["xbt"].rearrange("(t p) c -> t p c", p=P)      # [8, 128, 1536]
    wv_t = ap["wv"].rearrange("(t p) f -> t p f", p=P)
    out_t = ap["out"].rearrange("(t p) f -> t p f", p=P)

    # ---- persistent SBUF tiles
    qt_pool = ctx.enter_context(tc.tile_pool(name="qt", bufs=1))
    QT = [qt_pool.tile([P, 1024], bf16, name=f"qt{k}", tag=f"qt{k}") for k in range(KT)]
    acc_pool = ctx.enter_context(tc.tile_pool(name="acc", bufs=1))
    OACC = [acc_pool.tile([P, E], f32, name=f"oacc{t}", tag=f"oacc{t}") for t in range(NQT)]
    RS = [acc_pool.tile([P, NBLK], f32, name=f"rs{t}", tag=f"rs{t}") for t in range(NQT)]
    const_pool = ctx.enter_context(tc.tile_pool(name="const", bufs=1))
    fin_pool = ctx.enter_context(tc.tile_pool(name="fin", bufs=4))

    # Packed input streams, all on ONE queue in exact consumption order so
    # arrivals are strictly ordered: [wk_lo|x_blk0], wk_hi, wv, [xq|wq]
    # halves, constants, then x blocks 1-3. The e-outer projection loops
    # below consume each arriving tile completely (4 concurrent PSUM
    # accumulators) so the PE chases the DMA stream without group stalls.
    wkxa_pool = ctx.enter_context(tc.tile_pool(name="wkxap", bufs=1))
    wkxa = [wkxa_pool.tile([P, 1024], bf16, name=f"wkxa{e}", tag=f"wkxa{e}")
            for e in range(ET)]
    wkxb_pool = ctx.enter_context(tc.tile_pool(name="wkxbp", bufs=1))
    wkxb = [wkxb_pool.tile([P, 512], bf16, name=f"wkxb{e}", tag=f"wkxb{e}")
            for e in range(ET)]
    wv_pool = ctx.enter_context(tc.tile_pool(name="wvp", bufs=1))
    wv = [wv_pool.tile([P, E], bf16, name=f"wv{e}", tag=f"wv{e}") for e in range(ET)]
    xqwqa_pool = ctx.enter_context(tc.tile_pool(name="xqwqap", bufs=1))
    xqwqa = [xqwqa_pool.tile([P, 1024], bf16, name=f"xqwqa{e}", tag=f"xqwqa{e}")
             for e in range(ET)]
    xqwqb_pool = ctx.enter_context(tc.tile_pool(name="xqwqbp", bufs=1))
    xqwqb = [xqwqb_pool.tile([P, 1024], bf16, name=f"xqwqb{e}", tag=f"xqwqb{e}")
             for e in range(ET)]
    xbt_pool = ctx.enter_context(tc.tile_pool(name="xbtp", bufs=1))
    xbt = [xbt_pool.tile([P, 1536], bf16, name=f"xbt{e}", tag=f"xbt{e}")
           for e in range(ET)]

    def wkslice(e, k):
        # wk[e][:, k*128:(k+1)*128]: k 0-3 in wkxa, 4-7 in wkxb
        if k < 4:
            return wkxa[e][:, k * P:(k + 1) * P]
        return wkxb[e][:, (k - 4) * P:(k - 3) * P]

    def wqslice(e, k):
        # wq[e][:, k*128:(k+1)*128]: k 0-3 in xqwqa[512:], 4-7 in xqwqb[512:]
        src = xqwqa if k < 4 else xqwqb
        kk = k % 4
        return src[e][:, 512 + kk * P: 512 + (kk + 1) * P]

    def xqslice(e, qb):
        # xq[e][:, qb*512:(qb+1)*512]
        return (xqwqa if qb == 0 else xqwqb)[e][:, 0:512]

    def xblk(b, e, lo=0, hi=512):
        # x columns [lo, hi) within 512-key block b, of e-tile e
        if b == 0:
            return wkxa[e][:, 512 + lo:512 + hi]
        return xbt[e][:, (b - 1) * 512 + lo: (b - 1) * 512 + hi]

    for e in range(ET):
        nc.sync.dma_start(out=wkxa[e], in_=wkxa_t[e])
        if e >= 4:
            nc.sync.dma_start(out=wkxb[e - 4], in_=wkxb_t[e - 4])
    for e in range(4, ET):
        nc.sync.dma_start(out=wkxb[e], in_=wkxb_t[e])
        nc.sync.dma_start(out=wv[e - 4], in_=wv_t[e - 4])
    for e in range(4, ET):
        nc.sync.dma_start(out=wv[e], in_=wv_t[e])
        nc.sync.dma_start(out=xqwqa[e - 4], in_=xqwqa_t[e - 4])
    for e in range(4, ET):
        nc.sync.dma_start(out=xqwqa[e], in_=xqwqa_t[e])
        nc.sync.dma_start(out=xqwqb[e - 4], in_=xqwqb_t[e - 4])
    for e in range(4, ET):
        nc.sync.dma_start(out=xqwqb[e], in_=xqwqb_t[e])
    cm = const_pool.tile([P, 256], f32, name="cm")
    nc.sync.dma_start(out=cm, in_=ap["cmaskT"])
    ones = const_pool.tile([P, 1], bf16, name="ones")
    nc.sync.dma_start(out=ones, in_=ap["ones"])
    for e in range(ET):
        nc.sync.dma_start(out=xbt[e], in_=xbt_t[e])

    # ---- PSUM pools: pp 4 tags x 1 buf (8KB) + sp 2 bufs (4KB) + vp 2 tags
    # (4KB) = 16KB exactly. Softmax denominators reuse a corner of the
    # scores tile after exp has consumed it, so they need no pool of their
    # own.
    pp = ctx.enter_context(tc.tile_pool(name="pp", bufs=1, space="PSUM"))
    sp = ctx.enter_context(tc.tile_pool(name="sp", bufs=2, space="PSUM"))
    vp = ctx.enter_context(tc.tile_pool(name="vp", bufs=1, space="PSUM"))

    def eouter_proj(n_groups, lhs_of, rhs_of, evict_of, n_conc=4):
        # generic e-outer projection with n_conc concurrent accumulators:
        # each newly-arrived e-tile is consumed for all open groups at once.
        # Head projections (chasing the input DMA stream) use all 4 pp tags;
        # later ones use only pp0/pp1, leaving pp2/pp3 for the PV rotation.
        for gh in range(0, n_groups, n_conc):
            nj = min(n_conc, n_groups - gh)
            pss = [pp.tile([P, 512], f32, name=f"ps{j}", tag=f"pp{j}")
                   for j in range(nj)]
            for e in range(ET):
                for j in range(nj):
                    nc.tensor.matmul(pss[j], lhs_of(e, gh + j), rhs_of(e, gh + j),
                                     start=(e == 0), stop=(e == ET - 1))
            for j in range(nj):
                evict_of(gh + j, pss[j])

    def emit_qproj():
        # groups g = qb*8 + k, ordered (qb0,klo),(qb0,khi),(qb1,klo),(qb1,khi)
        def evict(g, ps):
            qb, k = divmod(g, KT)
            if k % 2 == 0:
                nc.vector.tensor_copy(QT[k][:, qb * 512:(qb + 1) * 512], ps)
            else:
                nc.scalar.copy(QT[k][:, qb * 512:(qb + 1) * 512], ps)
        eouter_proj(16,
                    lambda e, g: wqslice(e, g % KT),
                    lambda e, g: xqslice(e, g // KT),
                    evict)

    # ---- per-block KT/V pools (double-buffered across blocks)
    kt_pool = ctx.enter_context(tc.tile_pool(name="ktp", bufs=2))
    vb_pool = ctx.enter_context(tc.tile_pool(name="vbp", bufs=2))
    pt_pool = ctx.enter_context(tc.tile_pool(name="ptp", bufs=3))

    last_visit = {t: t // 2 for t in range(NQT)}

    pv_count = [0]

    def emit_pv(pend):
        # one item behind the scores stream so exp latency hides under PE work
        pb, w, blk, t, vbt, sps = pend
        nst = w // P
        # softmax denominator: sum over keys (partition axis) via N=1 matmuls;
        # written into a corner of the already-consumed scores tile (exp has
        # read it by now), so no dedicated PSUM pool is needed
        dps = sps[:, 0:1]
        for st in range(nst):
            nc.tensor.matmul(dps, pb[:, st * P:(st + 1) * P], ones,
                             start=(st == 0), stop=(st == nst - 1))
        nc.vector.tensor_copy(RS[t][:, blk:blk + 1], dps)
        is_last = (blk == last_visit[t])
        if is_last:
            # emitted before the PV matmuls: the DVE finishes these during
            # the PV, keeping them off the end-of-kernel critical chain
            rsum = fin_pool.tile([P, 1], f32, name="rsum", tag="rsum")
            nc.vector.reduce_sum(rsum, RS[t][:, :blk + 1], axis=X)
            rinv = fin_pool.tile([P, 1], f32, name="rinv", tag="rinv")
            nc.vector.reciprocal(rinv, rsum)
        # PV psum rotates across the vp pool AND the (idle during attention)
        # projection pool tags -> effectively triple-buffered, so this item's
        # PV never waits on the previous item's OACC eviction
        vps = [vp.tile([P, 512], f32, name=f"vps{fb}", tag=f"vp{fb}")
               for fb in range(2)]
        for st in range(nst):
            for fb in range(2):
                nc.tensor.matmul(vps[fb], pb[:, st * P:(st + 1) * P],
                                 vbt[st][:, fb * 512:(fb + 1) * 512],
                                 start=(st == 0), stop=(st == nst - 1))
        for fb in range(2):
            dst = OACC[t][:, fb * 512:(fb + 1) * 512]
            if blk == 0:
                nc.vector.tensor_copy(dst, vps[fb])
            else:
                nc.vector.tensor_add(dst, dst, vps[fb])
            if is_last:
                # scale into a bf16 staging tile + store, per half, so the
                # output DMA overlaps the other half's scale
                ob = fin_pool.tile([P, 512], bf16, name=f"ob{fb}",
                                   tag=f"ob{fb}")
                nc.scalar.activation(ob, dst, Copy, scale=rinv)
                nc.sync.dma_start(out=out_t[t][:, fb * 512:(fb + 1) * 512],
                                  in_=ob)

    pending = None
    for blk in range(NBLK):
        # K^T projection for this block: ktb[k] = [128 kd, 512 s]
        ktb = [kt_pool.tile([P, 512], bf16, name=f"ktb{k}", tag=f"ktb{k}")
               for k in range(KT)]

        def kt_evict(k, ps):
            if k % 2 == 0:
                nc.vector.tensor_copy(ktb[k], ps)
            else:
                nc.scalar.copy(ktb[k], ps)
        eouter_proj(KT,
                    lambda e, k: wkslice(e, k),
                    lambda e, k, b=blk: xblk(b, e),
                    kt_evict)

        # V projection for this block: vbt[st] = [128 s, 1024 f];
        # groups g = st*2 + fb
        vbt = [vb_pool.tile([P, E], bf16, name=f"vb{st}", tag=f"vb{st}")
               for st in range(4)]

        def v_evict(g, ps):
            st, fb = divmod(g, 2)
            if fb == 0:
                nc.scalar.copy(vbt[st][:, 0:512], ps)
            else:
                nc.vector.tensor_copy(vbt[st][:, 512:1024], ps)
        eouter_proj(8,
                    lambda e, g, b=blk: xblk(b, e, (g // 2) * P, (g // 2 + 1) * P),
                    lambda e, g: wv[e][:, (g % 2) * 512:(g % 2 + 1) * 512],
                    v_evict)

        if blk == 0:
            # Q projection sits between block-0 KV projection and attention:
            # its 4MB of inputs stream in while the KV matmuls run.
            emit_qproj()

        # attention items for this block (descending t: the final item of the
        # kernel is then (t=6, blk=3) with w=256, shortening the tail drain)
        for t in range(NQT - 1, 2 * blk - 1, -1):
            w = min(512, 256 * (t + 1) - 512 * blk)
            nst = w // P
            is_diag = (blk == last_visit[t])
            sps = sp.tile([P, 512], f32, name="sps", tag="sp")
            for st in range(nst):
                dst = sps[:, st * P:(st + 1) * P]
                for k in range(KT):
                    nc.tensor.matmul(dst, ktb[k][:, st * P:(st + 1) * P],
                                     QT[k][:, t * P:(t + 1) * P],
                                     start=(k == 0), stop=(k == KT - 1))
            if is_diag:
                nc.vector.tensor_add(sps[:, w - 256:w], sps[:, w - 256:w], cm)
            pb = pt_pool.tile([P, 512], bf16, name="pb", tag="pb")
            nc.scalar.activation(pb[:, :w], sps[:, :w], Exp, scale=SCALE)
            if pending is not None:
                emit_pv(pending)
            pending = (pb, w, blk, t, vbt, sps)
    emit_pv(pending)


def build_program():
    if "nc" in _prog_cache:
        return _prog_cache["nc"]
    from contextlib import ExitStack
    from concourse import bacc, mybir
    import concourse.tile as tile

    nc = bacc.Bacc("TRN2", target_bir_lowering=False, debug=False,
                   num_devices=NCORES)
    f32 = mybir.dt.float32
    bf16 = mybir.dt.bfloat16
    ap = {
        "wkxa": nc.dram_tensor("wkxa", [E, 1024], bf16, kind="ExternalInput").ap(),
        "wkxb": nc.dram_tensor("wkxb", [E, 512], bf16, kind="ExternalInput").ap(),
        "xqwqa": nc.dram_tensor("xqwqa", [E, 1024], bf16, kind="ExternalInput").ap(),
        "xqwqb": nc.dram_tensor("xqwqb", [E, 1024], bf16, kind="ExternalInput").ap(),
        "xbt": nc.dram_tensor("xbt", [E, 1536], bf16, kind="ExternalInput").ap(),
        "wv": nc.dram_tensor("wv", [E, E], bf16, kind="ExternalInput").ap(),
        "cmaskT": nc.dram_tensor("cmaskT", [P, 256], f32, kind="ExternalInput").ap(),
        "ones": nc.dram_tensor("ones", [P, 1], bf16, kind="ExternalInput").ap(),
        "out": nc.dram_tensor("out", [1024, E], bf16, kind="ExternalOutput").ap(),
    }
    with tile.TileContext(nc) as tc:
        with ExitStack() as ctx:
            _build_body(ctx, tc, ap)
    nc.compile()
    _prog_cache["nc"] = nc
    return nc


def make_in_maps(x, W_q, W_k, W_v):
    from concourse import mybir
    bf16 = mybir.dt.np(mybir.dt.bfloat16)
    x = np.asarray(x, np.float32)
    wqT = np.ascontiguousarray(np.asarray(W_q, np.float32).T).astype(bf16)
    wkT = np.ascontiguousarray(np.asarray(W_k, np.float32).T).astype(bf16)
    wvT = np.ascontiguousarray(np.asarray(W_v, np.float32).T).astype(bf16)
    # transposed-layout causal masks for the last 256 keys of the diagonal
    # block: maskT[p, st*128 + q] with s_local = st*128 + p, unmasked iff
    # s_local <= q + 128 (h=0, g odd) / s_local <= q (h=1, g even)
    p = np.arange(P)[:, None]
    q = np.arange(P)[None, :]
    def mk(thresh_extra):
        m0 = np.where(p <= q + thresh_extra, 0.0, NEG)          # st 0
        m1 = np.where(p + 128 <= q + thresh_extra, 0.0, NEG)    # st 1
        return np.concatenate([m0, m1], axis=1).astype(np.float32)
    cmasksT = [mk(128), mk(0)]
    ones = np.ones((P, 1), dtype=bf16)
    in_maps = []
    for c in range(NCORES):
        b, h = c // 2, c % 2
        xT = np.ascontiguousarray(x[b].T)
        qtiles = [2 * t + (1 - h) for t in range(NQT)]
        qcols = np.concatenate([np.arange(g * P, (g + 1) * P) for g in qtiles])
        xq = xT[:, qcols].astype(bf16)
        xp = xT.astype(bf16)
        c = np.concatenate
        in_maps.append({
            "wkxa": np.ascontiguousarray(c([wkT[:, :512], xp[:, :512]], axis=1)),
            "wkxb": np.ascontiguousarray(wkT[:, 512:]),
            "xqwqa": np.ascontiguousarray(c([xq[:, :512], wqT[:, :512]], axis=1)),
            "xqwqb": np.ascontiguousarray(c([xq[:, 512:], wqT[:, 512:]], axis=1)),
            "xbt": np.ascontiguousarray(xp[:, 512:]),
            "wv": wvT, "cmaskT": cmasksT[h], "ones": ones,
        })
    return in_maps


def assemble(results):
    out = np.zeros((B, S, E), np.float32)
    for c in range(NCORES):
        b, h = c // 2, c % 2
        co = np.asarray(results[c]["out"], dtype=np.float32)
        for t in range(NQT):
            g = 2 * t + (1 - h)
            out[b, g * P:(g + 1) * P, :] = co[t * P:(t + 1) * P]
    return out


def kernel(x, W_q, W_k, W_v):
    from concourse.bass_utils import run_bass_kernel_spmd
    nc = build_program()
    in_maps = make_in_maps(x, W_q, W_k, W_v)
    res = run_bass_kernel_spmd(nc, in_maps, core_ids=list(range(NCORES)))
    return assemble(res.results)


# revision 54
# speedup vs baseline: 2.6333x; 1.0100x over previous
"""No-collective causal-attention kernel: full local K/V recompute per core.

Core c = (batch c//2, parity h = c%2). Each core owns the interleaved query
tiles g = 2t + (1-h) (t = 0..7 local) of its batch, projects Q for those
rows, and projects the FULL K^T and V locally (no pair exchange) so the 8
cores run fully independent SPMD programs. The ~2.1G extra MACs per core are
far cheaper than the collective they replace.

Attention uses a transposed-scores formulation: S^T[s, q] tiles come straight
out of the scores matmul with keys on partitions, so exp(S^T) = P^T is
directly the stationary operand for the P@V matmuls — no PE transposes, no
PSUM->SBUF transpose copies. Softmax denominators are computed with N=1
matmuls against a ones vector (sum over the key/partition axis).

All operands are stored bf16 (fp32 PSUM accumulation), halving DMA and SBUF
and keeping every matmul at 1 cycle/row even at N=128.
"""

import numpy as np

B, S, E, KD = 4, 2048, 1024, 1024
NCORES = 8
P = 128
ET = E // P          # 8 e-tiles of the contraction dim
KT = KD // P         # 8 kd-tiles
NQT = 8              # local query tiles per core (128 rows each)
NBLK = 4             # 512-key blocks
NEG = -30000.0
SCALE = 1.0 / float(np.sqrt(KD))

_prog_cache = {}


def _build_body(ctx, tc, ap):
    from concourse import mybir

    nc = tc.nc
    f32 = mybir.dt.float32
    bf16 = mybir.dt.bfloat16
    Exp = mybir.ActivationFunctionType.Exp
    Copy = mybir.ActivationFunctionType.Copy
    X = mybir.AxisListType.X

    wkxa_t = ap["wkxa"].rearrange("(t p) c -> t p c", p=P)    # [8, 128, 1024]
    wkxb_t = ap["wkxb"].rearrange("(t p) c -> t p c", p=P)    # [8, 128, 512]
    xqwqa_t = ap["xqwqa"].rearrange("(t p) c -> t p c", p=P)  # [8, 128, 1024]
    xqwqb_t = ap["xqwqb"].rearrange("(t p) c -> t p c", p=P)  # [8, 128, 1024]
    xbt_t = ap["xbt"].rearrange("(t p) c -> t p c", p=P)      # [8, 128, 1536]
    wv_t = ap["wv"].rearrange("(t p) f -> t p f", p=P)
    out_t = ap["out"].rearrange("(t p) f -> t p f", p=P)

    # ---- persistent SBUF tiles
    qt_pool = ctx.enter_context(tc.tile_pool(name="qt", bufs=1))
    QT = [qt_pool.tile([P, 1024], bf16, name=f"qt{k}", tag=f"qt{k}") for k in range(KT)]
    acc_pool = ctx.enter_context(tc.tile_pool(name="acc", bufs=1))
    OACC = [acc_pool.tile([P, E], f32, name=f"oacc{t}", tag=f"oacc{t}") for t in range(NQT)]
    RS = [acc_pool.tile([P, NBLK], f32, name=f"rs{t}", tag=f"rs{t}") for t in range(NQT)]
    const_pool = ctx.enter_context(tc.tile_pool(name="const", bufs=1))
    fin_pool = ctx.enter_context(tc.tile_pool(name="fin", bufs=4))

    # Packed input streams, all on ONE queue in exact consumption order so
    # arrivals are strictly ordered: [wk_lo|x_blk0], wk_hi, wv, [xq|wq]
    # halves, constants, then x blocks 1-3. The e-outer projection loops
    # below consume each arriving tile completely (4 concurrent PSUM
    # accumulators) so the PE chases the DMA stream without group stalls.
    wkxa_pool = ctx.enter_context(tc.tile_pool(name="wkxap", bufs=1))
    wkxa = [wkxa_pool.tile([P, 1024], bf16, name=f"wkxa{e}", tag=f"wkxa{e}")
            for e in range(ET)]
    wkxb_pool = ctx.enter_context(tc.tile_pool(name="wkxbp", bufs=1))
    wkxb = [wkxb_pool.tile([P, 512], bf16, name=f"wkxb{e}", tag=f"wkxb{e}")
            for e in range(ET)]
    wv_pool = ctx.enter_context(tc.tile_pool(name="wvp", bufs=1))
    wv = [wv_pool.tile([P, E], bf16, name=f"wv{e}", tag=f"wv{e}") for e in range(ET)]
    xqwqa_pool = ctx.enter_context(tc.tile_pool(name="xqwqap", bufs=1))
    xqwqa = [xqwqa_pool.tile([P, 1024], bf16, name=f"xqwqa{e}", tag=f"xqwqa{e}")
             for e in range(ET)]
    xqwqb_pool = ctx.enter_context(tc.tile_pool(name="xqwqbp", bufs=1))
    xqwqb = [xqwqb_pool.tile([P, 1024], bf16, name=f"xqwqb{e}", tag=f"xqwqb{e}")
             for e in range(ET)]
    xbt_pool = ctx.enter_context(tc.tile_pool(name="xbtp", bufs=1))
    xbt = [xbt_pool.tile([P, 1536], bf16, name=f"xbt{e}", tag=f"xbt{e}")
           for e in range(ET)]

    def wkslice(e, k):
        # wk[e][:, k*128:(k+1)*128]: k 0-3 in wkxa, 4-7 in wkxb
        if k < 4:
            return wkxa[e][:, k * P:(k + 1) * P]
        return wkxb[e][:, (k - 4) * P:(k - 3) * P]

    def wqslice(e, k):
        # wq[e][:, k*128:(k+1)*128]: k 0-3 in xqwqa[512:], 4-7 in xqwqb[512:]
        src = xqwqa if k < 4 else xqwqb
        kk = k % 4
        return src[e][:, 512 + kk * P: 512 + (kk + 1) * P]

    def xqslice(e, qb):
        # xq[e][:, qb*512:(qb+1)*512]
        return (xqwqa if qb == 0 else xqwqb)[e][:, 0:512]

    def xblk(b, e, lo=0, hi=512):
        # x columns [lo, hi) within 512-key block b, of e-tile e
        if b == 0:
            return wkxa[e][:, 512 + lo:512 + hi]
        return xbt[e][:, (b - 1) * 512 + lo: (b - 1) * 512 + hi]

    for e in range(ET):
        nc.sync.dma_start(out=wkxa[e], in_=wkxa_t[e])
    for e in range(ET):
        nc.sync.dma_start(out=wkxb[e], in_=wkxb_t[e])
    for e in range(ET):
        nc.sync.dma_start(out=wv[e], in_=wv_t[e])
    for e in range(ET):
        nc.sync.dma_start(out=xqwqa[e], in_=xqwqa_t[e])
    for e in range(ET):
        nc.sync.dma_start(out=xqwqb[e], in_=xqwqb_t[e])
    cm = const_pool.tile([P, 256], f32, name="cm")
    nc.sync.dma_start(out=cm, in_=ap["cmaskT"])
    ones = const_pool.tile([P, 1], bf16, name="ones")
    nc.sync.dma_start(out=ones, in_=ap["ones"])
    for e in range(ET):
        nc.sync.dma_start(out=xbt[e], in_=xbt_t[e])

    # ---- PSUM pools: pp 4 tags x 1 buf (8KB) + sp 2 bufs (4KB) + vp 2 tags
    # (4KB) = 16KB exactly. Softmax denominators reuse a corner of the
    # scores tile after exp has consumed it, so they need no pool of their
    # own.
    pp = ctx.enter_context(tc.tile_pool(name="pp", bufs=1, space="PSUM"))
    sp = ctx.enter_context(tc.tile_pool(name="sp", bufs=2, space="PSUM"))
    vp = ctx.enter_context(tc.tile_pool(name="vp", bufs=1, space="PSUM"))

    def eouter_proj(n_groups, lhs_of, rhs_of, evict_of, n_conc=4):
        # generic e-outer projection with n_conc concurrent accumulators:
        # each newly-arrived e-tile is consumed for all open groups at once.
        # Head projections (chasing the input DMA stream) use all 4 pp tags;
        # later ones use only pp0/pp1, leaving pp2/pp3 for the PV rotation.
        for gh in range(0, n_groups, n_conc):
            nj = min(n_conc, n_groups - gh)
            pss = [pp.tile([P, 512], f32, name=f"ps{j}", tag=f"pp{j}")
                   for j in range(nj)]
            for e in range(ET):
                for j in range(nj):
                    nc.tensor.matmul(pss[j], lhs_of(e, gh + j), rhs_of(e, gh + j),
                                     start=(e == 0), stop=(e == ET - 1))
            for j in range(nj):
                evict_of(gh + j, pss[j])

    def emit_qproj():
        # groups g = qb*8 + k, ordered (qb0,klo),(qb0,khi),(qb1,klo),(qb1,khi)
        def evict(g, ps):
            qb, k = divmod(g, KT)
            if k % 2 == 0:
                nc.vector.tensor_copy(QT[k][:, qb * 512:(qb + 1) * 512], ps)
            else:
                nc.scalar.copy(QT[k][:, qb * 512:(qb + 1) * 512], ps)
        eouter_proj(16,
                    lambda e, g: wqslice(e, g % KT),
                    lambda e, g: xqslice(e, g // KT),
                    evict)

    # ---- per-block KT/V pools (double-buffered across blocks)
    kt_pool = ctx.enter_context(tc.tile_pool(name="ktp", bufs=2))
    vb_pool = ctx.enter_context(tc.tile_pool(name="vbp", bufs=2))
    pt_pool = ctx.enter_context(tc.tile_pool(name="ptp", bufs=3))

    last_visit = {t: t // 2 for t in range(NQT)}

    pv_count = [0]

    def emit_pv(pend):
        # one item behind the scores stream so exp latency hides under PE work
        pb, w, blk, t, vbt, sps = pend
        nst = w // P
        # softmax denominator: sum over keys (partition axis) via N=1 matmuls;
        # written into a corner of the already-consumed scores tile (exp has
        # read it by now), so no dedicated PSUM pool is needed
        dps = sps[:, 0:1]
        for st in range(nst):
            nc.tensor.matmul(dps, pb[:, st * P:(st + 1) * P], ones,
                             start=(st == 0), stop=(st == nst - 1))
        nc.vector.tensor_copy(RS[t][:, blk:blk + 1], dps)
        is_last = (blk == last_visit[t])
        if is_last:
            # emitted before the PV matmuls: the DVE finishes these during
            # the PV, keeping them off the end-of-kernel critical chain
            rsum = fin_pool.tile([P, 1], f32, name="rsum", tag="rsum")
            nc.vector.reduce_sum(rsum, RS[t][:, :blk + 1], axis=X)
            rinv = fin_pool.tile([P, 1], f32, name="rinv", tag="rinv")
            nc.vector.reciprocal(rinv, rsum)
        # PV psum rotates across the vp pool AND the (idle during attention)
        # projection pool tags -> effectively triple-buffered, so this item's
        # PV never waits on the previous item's OACC eviction
        vps = [vp.tile([P, 512], f32, name=f"vps{fb}", tag=f"vp{fb}")
               for fb in range(2)]
        for st in range(nst):
            for fb in range(2):
                nc.tensor.matmul(vps[fb], pb[:, st * P:(st + 1) * P],
                                 vbt[st][:, fb * 512:(fb + 1) * 512],
                                 start=(st == 0), stop=(st == nst - 1))
        for fb in range(2):
            dst = OACC[t][:, fb * 512:(fb + 1) * 512]
            if blk == 0:
                nc.vector.tensor_copy(dst, vps[fb])
            else:
                nc.vector.tensor_add(dst, dst, vps[fb])
            if is_last:
                # scale into a bf16 staging tile + store, per half, so the
                # output DMA overlaps the other half's scale
                ob = fin_pool.tile([P, 512], bf16, name=f"ob{fb}",
                                   tag=f"ob{fb}")
                nc.scalar.activation(ob, dst, Copy, scale=rinv)
                nc.sync.dma_start(out=out_t[t][:, fb * 512:(fb + 1) * 512],
                                  in_=ob)

    pending = None
    for blk in range(NBLK):
        # K^T projection for this block: ktb[k] = [128 kd, 512 s]
        ktb = [kt_pool.tile([P, 512], bf16, name=f"ktb{k}", tag=f"ktb{k}")
               for k in range(KT)]

        def kt_evict(k, ps):
            if k % 2 == 0:
                nc.vector.tensor_copy(ktb[k], ps)
            else:
                nc.scalar.copy(ktb[k], ps)
        eouter_proj(KT,
                    lambda e, k: wkslice(e, k),
                    lambda e, k, b=blk: xblk(b, e),
                    kt_evict)

        # V projection for this block: vbt[st] = [128 s, 1024 f];
        # groups g = st*2 + fb
        vbt = [vb_pool.tile([P, E], bf16, name=f"vb{st}", tag=f"vb{st}")
               for st in range(4)]

        def v_evict(g, ps):
            st, fb = divmod(g, 2)
            if fb == 0:
                nc.scalar.copy(vbt[st][:, 0:512], ps)
            else:
                nc.vector.tensor_copy(vbt[st][:, 512:1024], ps)
        eouter_proj(8,
                    lambda e, g, b=blk: xblk(b, e, (g // 2) * P, (g // 2 + 1) * P),
                    lambda e, g: wv[e][:, (g % 2) * 512:(g % 2 + 1) * 512],
                    v_evict)

        if blk == 0:
            # Q projection sits between block-0 KV projection and attention:
            # its 4MB of inputs stream in while the KV matmuls run.
            emit_qproj()

        # attention items for this block (ascending t: the small first item
        # fills the PV pipeline quickly after each block transition)
        for t in range(2 * blk, NQT):
            w = min(512, 256 * (t + 1) - 512 * blk)
            nst = w // P
            is_diag = (blk == last_visit[t])
            sps = sp.tile([P, 512], f32, name="sps", tag="sp")
            for st in range(nst):
                dst = sps[:, st * P:(st + 1) * P]
                for k in range(KT):
                    nc.tensor.matmul(dst, ktb[k][:, st * P:(st + 1) * P],
                                     QT[k][:, t * P:(t + 1) * P],
                                     start=(k == 0), stop=(k == KT - 1))
            if is_diag:
                nc.vector.tensor_add(sps[:, w - 256:w], sps[:, w - 256:w], cm)
            pb = pt_pool.tile([P, 512], bf16, name="pb", tag="pb")
            nc.scalar.activation(pb[:, :w], sps[:, :w], Exp, scale=SCALE)
            if pending is not None:
                emit_pv(pending)
            pending = (pb, w, blk, t, vbt, sps)
    emit_pv(pending)


def build_program():
    if "nc" in _prog_cache:
        return _prog_cache["nc"]
    from contextlib import ExitStack
    from concourse import bacc, mybir
    import concourse.tile as tile

    nc = bacc.Bacc("TRN2", target_bir_lowering=False, debug=False,
                   num_devices=NCORES)
    f32 = mybir.dt.float32
    bf16 = mybir.dt.bfloat16
    ap = {
        "wkxa": nc.dram_tensor("wkxa", [E, 1024], bf16, kind="ExternalInput").ap(),
        "wkxb": nc.dram_tensor("wkxb", [E, 512], bf16, kind="ExternalInput").ap(),
        "xqwqa": nc.dram_tensor("xqwqa", [E, 1024], bf16, kind="ExternalInput").ap(),
        "xqwqb": nc.dram_tensor("xqwqb", [E, 1024], bf16, kind="ExternalInput").ap(),
        "xbt": nc.dram_tensor("xbt", [E, 1536], bf16, kind="ExternalInput").ap(),
        "wv": nc.dram_tensor("wv", [E, E], bf16, kind="ExternalInput").ap(),
        "cmaskT": nc.dram_tensor("cmaskT", [P, 256], f32, kind="ExternalInput").ap(),
        "ones": nc.dram_tensor("ones", [P, 1], bf16, kind="ExternalInput").ap(),
        "out": nc.dram_tensor("out", [1024, E], bf16, kind="ExternalOutput").ap(),
    }
    with tile.TileContext(nc) as tc:
        with ExitStack() as ctx:
            _build_body(ctx, tc, ap)
    nc.compile()
    _prog_cache["nc"] = nc
    return nc


def make_in_maps(x, W_q, W_k, W_v):
    from concourse import mybir
    bf16 = mybir.dt.np(mybir.dt.bfloat16)
    x = np.asarray(x, np.float32)
    wqT = np.ascontiguousarray(np.asarray(W_q, np.float32).T).astype(bf16)
    wkT = np.ascontiguousarray(np.asarray(W_k, np.float32).T).astype(bf16)
    wvT = np.ascontiguousarray(np.asarray(W_v, np.float32).T).astype(bf16)
    # transposed-layout causal masks for the last 256 keys of the diagonal
    # block: maskT[p, st*128 + q] with s_local = st*128 + p, unmasked iff
    # s_local <= q + 128 (h=0, g odd) / s_local <= q (h=1, g even)
    p = np.arange(P)[:, None]
    q = np.arange(P)[None, :]
    def mk(thresh_extra):
        m0 = np.where(p <= q + thresh_extra, 0.0, NEG)          # st 0
        m1 = np.where(p + 128 <= q + thresh_extra, 0.0, NEG)    # st 1
        return np.concatenate([m0, m1], axis=1).astype(np.float32)
    cmasksT = [mk(128), mk(0)]
    ones = np.ones((P, 1), dtype=bf16)
    in_maps = []
    for c in range(NCORES):
        b, h = c // 2, c % 2
        xT = np.ascontiguousarray(x[b].T)
        qtiles = [2 * t + (1 - h) for t in range(NQT)]
        qcols = np.concatenate([np.arange(g * P, (g + 1) * P) for g in qtiles])
        xq = xT[:, qcols].astype(bf16)
        xp = xT.astype(bf16)
        c = np.concatenate
        in_maps.append({
            "wkxa": np.ascontiguousarray(c([wkT[:, :512], xp[:, :512]], axis=1)),
            "wkxb": np.ascontiguousarray(wkT[:, 512:]),
            "xqwqa": np.ascontiguousarray(c([xq[:, :512], wqT[:, :512]], axis=1)),
            "xqwqb": np.ascontiguousarray(c([xq[:, 512:], wqT[:, 512:]], axis=1)),
            "xbt": np.ascontiguousarray(xp[:, 512:]),
            "wv": wvT, "cmaskT": cmasksT[h], "ones": ones,
        })
    return in_maps


def assemble(results):
    out = np.zeros((B, S, E), np.float32)
    for c in range(NCORES):
        b, h = c // 2, c % 2
        co = np.asarray(results[c]["out"], dtype=np.float32)
        for t in range(NQT):
            g = 2 * t + (1 - h)
            out[b, g * P:(g + 1) * P, :] = co[t * P:(t + 1) * P]
    return out


def kernel(x, W_q, W_k, W_v):
    from concourse.bass_utils import run_bass_kernel_spmd
    nc = build_program()
    in_maps = make_in_maps(x, W_q, W_k, W_v)
    res = run_bass_kernel_spmd(nc, in_maps, core_ids=list(range(NCORES)))
    return assemble(res.results)
